# revision 13
# baseline (speedup 1.0000x reference)
"""2-layer GCN (GCNConv 128->128->64, N=50000, E=800000) on 8 TRN2 NeuronCores.

v2 strategy (dst-sharded, aggregate-first, chunk-pipelined AllGather):
  out = relu(A_hat @ relu(A_hat @ x @ W1 + b1) @ W2 + b2),  A_hat = D^-1/2 (A+I) D^-1/2
  - Per-edge norm dis[src]*dis[dst] folded into one per-edge scalar (esc);
    self-loop term dis[n]^2 via a diagonal matmul per 128-node block.
  - Feature tables are RAW bf16 features in wrapped-padded layout, split in
    K chunks by node-block quarter so gather row indices fit int16.
    Layer 1's table is host-prepared from x and replicated (ExternalInput) —
    no device-side table build and no first AllGather at all.
  - Edges sorted by (dst-quarter Q, src-chunk c, dst-block b), padded to
    128-edge tiles (max over cores so the SPMD program is uniform).
  - Per (Q,c): one batched dma_gather run (chunks of <=GCAP tiles), batched
    one-hot build (iota==dst_rel) + per-edge scale on VectorE, PSUM-matmul
    scatter-add into per-block accumulators packed 4-per-PSUM-bank (bank
    zero-init matmul sets has_written bits so slices accumulate safely).
  - Epilogue per block: agg[ch,dst] -> (lhsT=agg)@W + ones(x)bias-row matmul
    -> relu on ScalarE directly in [dst,ch] layout (no transposes).
  - Layer 1 fires the AllGather of each quarter's h1 shard right after that
    quarter's epilogues, so K-1 of the K AllGathers overlap layer-1 compute;
    layer 2 consumes chunk c only after AllGather c (tile deps handle it).
    AG outputs use addr_space="Shared" (fast ncfw path).
"""

import numpy as np
import ml_dtypes

import concourse.bass as bass
import concourse.bacc as bacc
import concourse.mybir as mybir
import concourse.tile as tile
from concourse.bass_utils import run_bass_kernel_spmd
from concourse.library_config import mlp
from concourse.masks import make_identity

P = 128
N_NODES = 50000
N_EDGES = 800000
IN_CH = 128
HID_CH = 128
OUT_CH = 64
N_CORES = 8
NSH = N_NODES // N_CORES          # 6250 nodes per core
NBLK = (NSH + P - 1) // P         # 49 blocks per core (48 full + 106 tail)
NFULL = NSH // P                  # 48
NTAIL = NSH - NFULL * P           # 106

K = 4                             # node-quarter chunks (AG granularity)
QSPLIT = [0, 13, 25, 37, 49]      # block ranges per quarter
NBLKC = [QSPLIT[i + 1] - QSPLIT[i] for i in range(K)]
GCAP = 8                          # tiles (x128 idxs) per dma_gather call
SCRATCH = 16384                   # SWDGE descriptor ring carveout (bytes/partition)

BF16 = mybir.dt.bfloat16
F32 = mybir.dt.float32

LAST_RESULT = None  # for test harness: BassKernelResults of last run


def _chunks(t, cap=GCAP):
    """Split t tiles into balanced chunks of <= cap (e.g. 17 -> 9+8)."""
    if t == 0:
        return []
    n = -(-t // cap)
    base, rem = divmod(t, n)
    return [base + (1 if i < rem else 0) for i in range(n)]


def _host_prep(edge_index):
    """Index-only preprocessing. Returns per-core upload arrays + tile plan."""
    src = edge_index[0].astype(np.int64)
    dst = edge_index[1].astype(np.int64)

    deg = np.bincount(dst, minlength=N_NODES) + 1      # self-loop included
    dis = (1.0 / np.sqrt(deg.astype(np.float64))).astype(np.float32)

    qof = np.repeat(np.arange(K), NBLKC)               # block -> quarter

    kd = dst // NSH
    idd = dst - kd * NSH
    bd = idd // P                                      # dst block
    qd = qof[bd]                                       # dst quarter

    ks = src // NSH
    ids = src - ks * NSH
    rs = ids // P
    ps = ids - rs * P
    cs = qof[rs]                                       # src chunk
    qsplit = np.asarray(QSPLIT)
    nblkc = np.asarray(NBLKC)
    row_local = (ks * P + ps) * nblkc[cs] + (rs - qsplit[cs])  # < 1024*13 fits int16

    # group = (core, Q, c, b); edges sorted into that order
    gkey = ((kd * K + qd) * K + cs) * NBLK + bd
    order = np.argsort(gkey, kind="stable")
    g_sorted = gkey[order]

    n_groups = N_CORES * K * K * NBLK
    cnt = np.bincount(g_sorted, minlength=n_groups).reshape(N_CORES, K, K, NBLK)
    tmax = -(-cnt.max(axis=0) // P)                    # [K,K,NBLK] tiles per group

    # traversal order: q, c, b in quarter q
    tiles_qc = []        # [q][c] -> list of (b, ntiles)
    sect_tiles = []      # per traversed group
    group_sect = np.full(K * K * NBLK, -1, np.int64)   # (q,c,b) -> sect idx
    for q in range(K):
        row = []
        for c in range(K):
            runs = []
            for b in range(QSPLIT[q], QSPLIT[q + 1]):
                t = int(tmax[q, c, b])
                if t > 0:
                    runs.append((b, t))
                group_sect[(q * K + c) * NBLK + b] = len(sect_tiles)
                sect_tiles.append(t)
            row.append(runs)
        tiles_qc.append(row)
    sect_tiles = np.asarray(sect_tiles)
    T_total = int(sect_tiles.sum())
    sect_base = np.concatenate([[0], np.cumsum(sect_tiles)])[:-1] * P

    # per-edge slot within its core's padded edge list
    group_start = np.concatenate([[0], np.cumsum(cnt.reshape(-1))])[:-1]
    pos_in_group = np.arange(len(g_sorted)) - group_start[g_sorted]
    sect_idx = group_sect[g_sorted % (K * K * NBLK)]
    slot = sect_base[sect_idx] + pos_in_group
    edge_core = g_sorted // (K * K * NBLK)

    EPC = T_total * P
    idx_rows = np.zeros((N_CORES, EPC), np.int64)
    dst_rel = np.full((N_CORES, EPC), -1.0, np.float32)
    escv = np.zeros((N_CORES, EPC), np.float32)

    s_ord = order  # edge order
    idx_rows[edge_core, slot] = row_local[s_ord]
    dst_rel[edge_core, slot] = (idd[s_ord] - bd[s_ord] * P).astype(np.float32)
    escv[edge_core, slot] = dis[src[s_ord]] * dis[dst[s_ord]]

    # wrap indices: idx i -> [i%16, i//16], replicated to 128 partitions
    ii = np.arange(EPC)
    idxw = np.zeros((N_CORES, 16, T_total * 8), np.int16)
    for k in range(N_CORES):
        w = np.zeros((16, T_total * 8), np.int16)
        w[ii % 16, ii // 16] = idx_rows[k]
        idxw[k] = w
    idxw = np.tile(idxw, (1, 8, 1))                    # [N_CORES, 128, T*8]

    # scaled one-hot tiles: oh[core][p, t*128 + c] = escv if dst_rel==c else 0
    # (pure edge-structure; streamed from HBM, replaces on-device build)
    onehot = np.zeros((N_CORES, P, T_total, P), ml_dtypes.bfloat16)
    slots = np.arange(EPC)
    for k in range(N_CORES):
        dr = dst_rel[k].astype(np.int64)
        m = dr >= 0
        onehot[k, slots[m] % P, slots[m] // P, dr[m]] = escv[k, m].astype(
            ml_dtypes.bfloat16)
    onehot = onehot.reshape(N_CORES, P, T_total * P)

    # disw2: [128, NBLK] per core: dis^2 of own nodes (0 on tail padding)
    disw2 = np.zeros((N_CORES, P, NBLK), np.float32)
    nodes = np.arange(NBLK * P)
    valid = nodes < NSH
    for k in range(N_CORES):
        v = np.zeros(NBLK * P, np.float32)
        v[valid] = (dis[k * NSH + nodes[valid]] ** 2)
        disw2[k] = v.reshape(NBLK, P).T

    # diagonal self-loop matrices: dgall[p, b*128+c] = dis2[p,b] if c==p else 0
    dgall = np.zeros((N_CORES, P, NBLK, P), ml_dtypes.bfloat16)
    pp = np.arange(P)
    for k in range(N_CORES):
        dgall[k, pp, :, pp] = disw2[k].astype(ml_dtypes.bfloat16)[pp, :]
    dgall = dgall.reshape(N_CORES, P, NBLK * P)

    return {
        "tiles_qc": tiles_qc, "T_total": T_total,
        "idxw": idxw.astype(np.int16),
        "onehot": onehot,
        "dgall": dgall,
        # for host-side emulation / debugging:
        "idx_rows": idx_rows, "dst_rel": dst_rel, "escv": escv, "dis": dis,
        "disw2": disw2,
    }


def _build(plan):
    """Build the SPMD program (identical across cores)."""
    T_total = plan["T_total"]
    tiles_qc = plan["tiles_qc"]
    nc = bacc.Bacc("TRN2", target_bir_lowering=False, num_devices=N_CORES,
                   num_swdge_queues=4, dynamic_dma_scratch_size=SCRATCH)

    t_xtab = [nc.dram_tensor(f"xtab{c}", [N_CORES * P, NBLKC[c] * IN_CH], BF16,
                             kind="ExternalInput") for c in range(K)]
    t_x1st = nc.dram_tensor("x1st", [P, NBLK * IN_CH], BF16, kind="ExternalInput")
    t_w1 = nc.dram_tensor("w1", [IN_CH, HID_CH], BF16, kind="ExternalInput")
    t_b1 = nc.dram_tensor("b1r", [1, HID_CH], BF16, kind="ExternalInput")
    t_w2 = nc.dram_tensor("w2", [HID_CH, OUT_CH], BF16, kind="ExternalInput")
    t_b2 = nc.dram_tensor("b2r", [1, OUT_CH], BF16, kind="ExternalInput")
    t_dgall = nc.dram_tensor("dgall", [P, NBLK * P], BF16, kind="ExternalInput")
    t_idxw = nc.dram_tensor("idxw", [P, T_total * 8], mybir.dt.int16,
                            kind="ExternalInput")
    t_oh = nc.dram_tensor("oh", [P, T_total * P], BF16, kind="ExternalInput")
    t_out = nc.dram_tensor("out", [NSH, OUT_CH], F32, kind="ExternalOutput")

    x2sh = [nc.dram_tensor(f"x2sh{c}", [P, NBLKC[c] * HID_CH], BF16)
            for c in range(K)]
    x2f = [nc.dram_tensor(f"x2f{c}", [N_CORES * P, NBLKC[c] * HID_CH], BF16,
                          addr_space="Shared") for c in range(K)]

    rg = [list(range(N_CORES))]
    gq = [0]  # gather queue round-robin

    with tile.TileContext(nc) as tc:
        with (
            tc.tile_pool(name="const", bufs=1) as cp,
            tc.tile_pool(name="sbuf", bufs=3) as sb,
            tc.tile_pool(name="gpool", bufs=2) as gp,
            tc.tile_pool(name="psum", bufs=2, space="PSUM") as ps,
            tc.tile_pool(name="pacc", bufs=1, space="PSUM") as pa,
        ):
            nc.gpsimd.load_library(mlp)

            idx_sb = cp.tile([P, T_total * 8], mybir.dt.int16)
            nc.sync.dma_start(out=idx_sb[:], in_=t_idxw[:, :])

            x1stage = cp.tile([P, NBLK, IN_CH], BF16)
            nc.sync.dma_start(
                out=x1stage[:],
                in_=t_x1st[:, :].rearrange("p (b c) -> p b c", c=IN_CH))

            dgall = cp.tile([P, NBLK, P], BF16)
            nc.sync.dma_start(
                out=dgall[:],
                in_=t_dgall[:, :].rearrange("p (b c) -> p b c", c=P))

            w1_sb = cp.tile([IN_CH, HID_CH], BF16)
            nc.sync.dma_start(out=w1_sb[:], in_=t_w1[:, :])
            w2_sb = cp.tile([HID_CH, OUT_CH], BF16)
            nc.sync.dma_start(out=w2_sb[:], in_=t_w2[:, :])
            b1row = cp.tile([1, HID_CH], BF16)
            nc.sync.dma_start(out=b1row[:], in_=t_b1[:, :])
            b2row = cp.tile([1, OUT_CH], BF16)
            nc.sync.dma_start(out=b2row[:], in_=t_b2[:, :])

            ones1 = cp.tile([1, P], BF16)
            nc.vector.memset(ones1[:], 1.0)
            zrow = cp.tile([1, 512], BF16)
            nc.vector.memset(zrow[:], 0.0)

            x2stage = cp.tile([P, NBLK, HID_CH], BF16)

            deferred = []  # AG emissions deferred into the next gather stream

            def layer(li):
                Tg = 0
                stage = x1stage if li == 0 else x2stage
                for q in range(K):
                    q0, q1 = QSPLIT[q], QSPLIT[q + 1]
                    nbanks = -(-(q1 - q0) // 4)
                    accs = [pa.tile([HID_CH, 512], F32, tag=f"acc{i}",
                                    name=f"acc{i}") for i in range(nbanks)]
                    # zero-init each bank (sets has_written for whole bank)
                    for a in accs:
                        nc.tensor.matmul(out=a[:], lhsT=ones1[:], rhs=zrow[:],
                                         start=True, stop=False)

                    def acc_ap(b):
                        i = b - q0
                        return accs[i // 4][:, (i % 4) * P : (i % 4 + 1) * P]

                    for c in range(K):
                        runs = tiles_qc[q][c]
                        R = sum(t for _, t in runs)
                        if R == 0:
                            continue
                        blk_of = []
                        for b, t in runs:
                            blk_of += [b] * t
                        table = t_xtab[c] if li == 0 else x2f[c]
                        tabv = table.ap().rearrange("q2 (r c2) -> (q2 r) c2",
                                                    c2=HID_CH)
                        lo = 0
                        for n in _chunks(R):
                            g = gp.tile([P, GCAP, HID_CH], BF16, tag="g")
                            nc.gpsimd.dma_gather(
                                out_ap=g[:, :n, :],
                                in_ap=tabv[0 : N_CORES * P * NBLKC[c], :],
                                idxs_ap=idx_sb[:, 8 * (Tg + lo) : 8 * (Tg + lo + n)],
                                num_idxs=n * P, num_idxs_reg=n * P,
                                elem_size=HID_CH, queue_num=gq[0] % 4,
                            )
                            gq[0] += 1
                            while deferred:
                                deferred.pop(0)()
                            oh = gp.tile([P, GCAP, P], BF16, tag="oh")
                            nc.sync.dma_start(
                                out=oh[:, :n, :],
                                in_=t_oh[:, (Tg + lo) * P : (Tg + lo + n) * P]
                                .rearrange("p (t c) -> p t c", c=P))
                            for j in range(n):
                                nc.tensor.matmul(
                                    out=acc_ap(blk_of[lo + j]),
                                    lhsT=g[:, j, :], rhs=oh[:, j, :],
                                    start=False, stop=False,
                                )
                            lo += n
                        Tg += R

                    for b in range(q0, q1):
                        nb = P if b < NFULL else NTAIL
                        nc.tensor.matmul(
                            out=acc_ap(b), lhsT=stage[:, b, :],
                            rhs=dgall[:, b, :], start=False, stop=True,
                        )
                        t_sb = sb.tile([HID_CH, P], BF16, tag="tsb")
                        nc.scalar.copy(out=t_sb[:], in_=acc_ap(b))
                        if li == 0:
                            ups = ps.tile([P, HID_CH], F32, tag="ups1")
                            nc.tensor.matmul(out=ups[:], lhsT=t_sb[:],
                                             rhs=w1_sb[:], start=True, stop=False)
                            nc.tensor.matmul(out=ups[:], lhsT=ones1[:],
                                             rhs=b1row[:], start=False, stop=True)
                            nc.scalar.activation(
                                out=x2stage[:, b, :], in_=ups[:],
                                func=mybir.ActivationFunctionType.Relu)
                        else:
                            ups = ps.tile([P, OUT_CH], F32, tag="ups2")
                            nc.tensor.matmul(out=ups[:], lhsT=t_sb[:],
                                             rhs=w2_sb[:], start=True, stop=False)
                            nc.tensor.matmul(out=ups[:], lhsT=ones1[:],
                                             rhs=b2row[:], start=False, stop=True)
                            outt = sb.tile([P, OUT_CH], F32, tag="outt")
                            nc.scalar.activation(
                                out=outt[:], in_=ups[:],
                                func=mybir.ActivationFunctionType.Relu)
                            nc.sync.dma_start(out=t_out[b * P : b * P + nb, :],
                                              in_=outt[:nb, :])

                    if li == 0:
                        nc.sync.dma_start(
                            out=x2sh[q][:, :],
                            in_=x2stage[:, q0:q1, :].rearrange("p b c -> p (b c)"))

                        def emit_ag(q=q):
                            nc.gpsimd.collective_compute(
                                "AllGather", mybir.AluOpType.bypass,
                                replica_groups=rg,
                                ins=[x2sh[q].ap().opt()],
                                outs=[x2f[q].ap().opt()],
                            )
                        deferred.append(emit_ag)

            layer(0)
            layer(1)
            assert not deferred

    nc.compile()
    return nc


def _make_in_maps(x, W1, b1, W2, b2, prep):
    """Per-core input dicts (xtab chunks replicated, index arrays per core)."""
    xb = np.ascontiguousarray(x).astype(ml_dtypes.bfloat16)
    wrap = np.zeros((N_CORES, NBLK, P, IN_CH), ml_dtypes.bfloat16)
    full = xb.reshape(N_CORES, NSH, IN_CH)
    wrap.reshape(N_CORES, NBLK * P, IN_CH)[:, :NSH] = full
    # xtab rows: k*128+p, cols r_local*128+ch
    xtab_full = np.ascontiguousarray(
        wrap.transpose(0, 2, 1, 3).reshape(N_CORES * P, NBLK * IN_CH))
    xtabs = [np.ascontiguousarray(
        xtab_full[:, QSPLIT[c] * IN_CH : QSPLIT[c + 1] * IN_CH])
        for c in range(K)]
    # x1 stage per core: [p, r*128+ch]
    x1st = np.ascontiguousarray(
        wrap.transpose(0, 2, 1, 3).reshape(N_CORES, P, NBLK * IN_CH))

    w1b = np.asarray(W1, np.float32).astype(ml_dtypes.bfloat16)
    w2b = np.asarray(W2, np.float32).astype(ml_dtypes.bfloat16)
    b1b = np.asarray(b1, np.float32).reshape(1, HID_CH).astype(ml_dtypes.bfloat16)
    b2b = np.asarray(b2, np.float32).reshape(1, OUT_CH).astype(ml_dtypes.bfloat16)

    in_maps = []
    for k in range(N_CORES):
        m = {f"xtab{c}": xtabs[c] for c in range(K)}
        m.update({
            "x1st": x1st[k], "w1": w1b, "b1r": b1b, "w2": w2b, "b2r": b2b,
            "dgall": np.ascontiguousarray(prep["dgall"][k]),
            "idxw": np.ascontiguousarray(prep["idxw"][k]),
            "oh": np.ascontiguousarray(prep["onehot"][k]),
        })
        in_maps.append(m)
    return in_maps


def kernel(x, edge_index, W1, b1, W2, b2, _trace=False):
    global LAST_RESULT
    x = np.asarray(x, dtype=np.float32)
    edge_index = np.asarray(edge_index, dtype=np.int32)

    prep = _host_prep(edge_index)
    nc = _build(prep)
    in_maps = _make_in_maps(x, W1, b1, W2, b2, prep)

    res = run_bass_kernel_spmd(nc, in_maps, core_ids=list(range(N_CORES)),
                               trace=_trace)
    LAST_RESULT = res
    out = np.concatenate([res.results[k]["out"] for k in range(N_CORES)], axis=0)
    return out.astype(np.float32)
